# revision 61
# baseline (speedup 1.0000x reference)
"""Bahdanau attention TRN2 Bass kernel.

Math (per batch b):
    q_proj = query @ w1 + b1                      # [U]
    v_proj = values[b] @ w2 + b2                  # [S, U]
    score  = tanh(q_proj + v_proj) @ wv + bv      # [S]
    aw     = softmax(score)                       # [S]
    ctx    = sum_s aw[s] * values[b, s, :]        # [D]

Sharding: batch B=32 split 4-per-core across 8 NeuronCores; weights replicated.

Per-core layout strategy (bf16 compute, fp32 accumulation):
  - values loaded fp32->bf16 (SWDGE cast DMA) into small staging tiles,
    bounced through a DRAM bf16 copy, re-read with DMA transpose into
    [d_part, s_free] tiles (vT) and re-read naturally into [s_part, d_free]
    tiles (vnat, used only by the context matmuls).
  - main matmul: psum[u(128), s(512)] += w2bf[d,u].T @ vT[d, s] over 8 d-chunks
  - tanh fused with bias add on ScalarE: bias = q_proj[u] + b1 + b2 per
    partition, broadcast along s.
  - score: psum[1, s] += wv[u].T @ tanh[u, s] accumulated over 8 u-chunks.
  - softmax on one partition ([1, 2048]); no max-subtraction needed (scores
    are bounded by |wv| * sqrt(U), far inside fp32 exp range).
  - aw transposed to [s_part, chunk] via 16 tiny K=1 matmuls against ones.
  - context: psum[1, d] += aw[s].T @ vnat[s, d] over 16 s-chunks.

The four batches are software-pipelined (mm(b), tail(b-1), load(b+1)) so each
batch's softmax latency chain and context matmuls overlap the next batch's
main matmuls; loads never hold long-lived SBUF slots so the DMA pipeline
never waits on compute.
"""

import numpy as np

import concourse.bass as bass
import concourse.mybir as mybir
import concourse.tile as tile
from concourse import bacc
from concourse.bass_utils import run_bass_kernel_spmd
from concourse.masks import make_identity

B, S, D, U = 32, 2048, 1024, 1024
N_CORES = 8
BPC = B // N_CORES  # batches per core
P = 128
DC = D // P  # 8 d-chunks
UC = U // P  # 8 u-chunks
SC = S // P  # 16 s-chunks
NSB = 4  # s-blocks per batch
SBS = S // NSB  # 512

f32 = mybir.dt.float32
bf16 = mybir.dt.bfloat16
AF = mybir.ActivationFunctionType
ALU = mybir.AluOpType
AX = mybir.AxisListType


def emit(nc: bass.Bass, tc: tile.TileContext, io: dict):
    vals, qry = io["vals"], io["qry"]
    w1, b1, w2, b2, wv, bv = io["w1"], io["b1"], io["w2"], io["b2"], io["wv"], io["bv"]
    ctx_out, aw_out = io["ctx_out"], io["aw_out"]

    with (
        tc.tile_pool(name="consts", bufs=1) as consts,
        tc.tile_pool(name="stage", bufs=2) as stage_pool,
        tc.tile_pool(name="vnat", bufs=2) as vn_pool,
        tc.tile_pool(name="vt", bufs=2) as vt_pool,
        tc.tile_pool(name="tanh", bufs=3) as tanh_pool,
        tc.tile_pool(name="small", bufs=2) as small,
        tc.tile_pool(name="dram", bufs=2, space="DRAM") as dram_pool,
        tc.tile_pool(name="psmm", bufs=2, space="PSUM") as psmm,
        tc.tile_pool(name="pssc", bufs=2, space="PSUM") as pssc,
        tc.tile_pool(name="psq", bufs=1, space="PSUM") as psq,
        tc.tile_pool(name="psawt", bufs=1, space="PSUM") as psawt,
        tc.tile_pool(name="psctx", bufs=2, space="PSUM") as psctx,
    ):
        # ---- small constants on the ACT HWDGE ring (SP ring and SWDGE stay
        # free for the values pipeline) ----
        w2bf = consts.tile([P, DC, U], bf16)
        wvbf = consts.tile([P, UC], bf16)
        wvf = consts.tile([P, UC], f32)
        b1s = consts.tile([P, UC], f32)
        b2s = consts.tile([P, UC], f32)
        bvs = consts.tile([1, 1], f32)
        ones1 = consts.tile([1, 1], bf16)

        def load_consts():
            with nc.allow_non_contiguous_dma(reason="tiny weight reshapes"):
                nc.scalar.dma_start(
                    out=wvf, in_=wv.rearrange("(uc up) o -> up (uc o)", up=P)
                )
                nc.scalar.dma_start(
                    out=b1s, in_=b1.rearrange("(uc up) -> up uc", up=P)
                )
                nc.scalar.dma_start(
                    out=b2s, in_=b2.rearrange("(uc up) -> up uc", up=P)
                )
                nc.scalar.dma_start(out=bvs, in_=bv.rearrange("(a o) -> a o", a=1))
            nc.vector.tensor_copy(out=wvbf, in_=wvf)
            nc.vector.memset(ones1, 1.0)

        def load_w2():
            nc.gpsimd.dma_start(
                out=w2bf, in_=w2.rearrange("(dc dp) u -> dp dc u", dp=P)
            )

        # ---- q_plus[u, b] = query @ w1 + b1 + b2 (tanh bias terms) ----
        # w1/query ride the ACT HWDGE ring as fp32 (cast on idle DVE) in
        # per-uc column blocks, so the q-proj matmuls start within ~2us and
        # never queue behind the SWDGE value-cast stream.
        q_plus = consts.tile([P, UC, BPC], f32)

        def setup_q(after_early=None):
            with tc.tile_pool(name="setup", bufs=1) as setup:
                # one contiguous 16KB load + PE transposes beats 8 strided
                # DMAs through the serialized DMA pipe at startup
                qnat = setup.tile([BPC, D], f32)
                nc.scalar.dma_start(out=qnat, in_=qry[:, :])
                id4 = setup.tile([BPC, BPC], f32)
                make_identity(nc, id4)
                # small consts go behind qnat so the first PE transpose isn't
                # stuck behind their slow strided descriptors
                load_consts()
                qTbf = setup.tile([P, DC, BPC], bf16)
                for dc in range(DC):
                    ps_t = psq.tile([P, BPC], f32, tag="ps")
                    nc.tensor.transpose(
                        ps_t, qnat[:, dc * P : (dc + 1) * P], id4
                    )
                    nc.vector.tensor_copy(out=qTbf[:, dc, :], in_=ps_t)
                w1_t = w1.rearrange("(dc dp) u -> dp dc u", dp=P)
                for uc in range(UC):
                    w1f = setup.tile([P, DC, P], f32, tag="w1f", bufs=2)
                    nc.scalar.dma_start(
                        out=w1f, in_=w1_t[:, :, uc * P : (uc + 1) * P]
                    )
                    w1bf = setup.tile([P, DC, P], bf16, tag="w1bf", bufs=2)
                    nc.vector.tensor_copy(out=w1bf, in_=w1f)
                    ps = psq.tile([P, BPC], f32)
                    for dc in range(DC):
                        nc.tensor.matmul(
                            ps,
                            lhsT=w1bf[:, dc, :],
                            rhs=qTbf[:, dc, :],
                            start=(dc == 0),
                            stop=(dc == DC - 1),
                        )
                    nc.vector.tensor_scalar(
                        out=q_plus[:, uc, :],
                        in0=ps,
                        scalar1=b1s[:, uc : uc + 1],
                        scalar2=b2s[:, uc : uc + 1],
                        op0=ALU.add,
                        op1=ALU.add,
                    )
                    if uc == 1 and after_early is not None:
                        # inject the head of the values pipeline here so its
                        # DMAs queue right behind the first two w1 blocks
                        after_early()

        # ---- per-batch stages, software-pipelined ----
        # PE program order: mm(0), mm(1), tail(0), mm(2), tail(1), mm(3),
        # tail(2), tail(3) — so each batch's softmax (ACT/DVE latency chain)
        # overlaps the next batch's main matmuls instead of stalling PE.
        state = {}

        def load(b, blocks=None):
            # fp32 -> bf16 cast into a small staging tile (SWDGE), bounce via
            # DRAM, and DMA-transpose back into vT. Nothing here holds a
            # long-lived SBUF slot, so batch b+1's load never waits on batch
            # b-1's context matmuls.
            if b not in state:
                state[b] = {
                    "vT": vt_pool.tile([P, DC, S], bf16, tag="vT", name=f"vT{b}"),
                    "vdram": dram_pool.tile(
                        [S, D], bf16, tag="vdram", name=f"vdram{b}"
                    ),
                    "writes": {},
                }
            vT = state[b]["vT"]
            vbf_dram = state[b]["vdram"]
            vals_b = vals[b].rearrange("(sc sp) d -> sp sc d", sp=P)
            vdram_t = vbf_dram.rearrange("(sc sp) d -> sp sc d", sp=P)
            ncb = SC // NSB  # s-chunks per block
            for sb in blocks if blocks is not None else range(NSB):
                sl = slice(sb * ncb, (sb + 1) * ncb)
                stg = stage_pool.tile([P, ncb, D], bf16, tag="stg", name=f"stg{b}_{sb}")
                nc.gpsimd.dma_start(out=stg, in_=vals_b[:, sl, :])
                w_inst = nc.sync.dma_start(out=vdram_t[:, sl, :], in_=stg)
                state[b]["writes"][sb] = w_inst
                rows = slice(sb * SBS, (sb + 1) * SBS)
                for dc in range(DC):
                    t_inst = nc.sync.dma_start_transpose(
                        out=vT[:, dc, rows],
                        in_=vbf_dram[rows, dc * P : (dc + 1) * P],
                    )

        def mm(b):
            vT = state[b]["vT"]
            escore = small.tile([1, S], f32, tag="escore")
            for sb in range(NSB):
                ssl = slice(sb * SBS, (sb + 1) * SBS)
                ps_s = pssc.tile([1, SBS], f32)
                for uc in range(UC):
                    ps_v = psmm.tile([P, SBS], f32)
                    for dc in range(DC):
                        nc.tensor.matmul(
                            ps_v,
                            lhsT=w2bf[:, dc, uc * P : (uc + 1) * P],
                            rhs=vT[:, dc, ssl],
                            start=(dc == 0),
                            stop=(dc == DC - 1),
                        )
                    th = tanh_pool.tile([P, SBS], bf16)
                    nc.scalar.activation(
                        out=th,
                        in_=ps_v,
                        func=AF.Tanh,
                        bias=q_plus[:, uc, b : b + 1],
                        scale=1.0,
                    )
                    nc.tensor.matmul(
                        ps_s,
                        lhsT=wvbf[:, uc : uc + 1],
                        rhs=th,
                        start=(uc == 0),
                        stop=(uc == UC - 1),
                        skip_group_check=True,
                    )
                nc.scalar.activation(
                    out=escore[0:1, ssl],
                    in_=ps_s,
                    func=AF.Exp,
                    bias=bvs[0:1, :],
                    scale=1.0,
                )

            # softmax normalization on a single partition (in-place on escore)
            denom = small.tile([1, 1], f32, tag="denom")
            nc.vector.reduce_sum(out=denom, in_=escore, axis=AX.X)
            rden = small.tile([1, 1], f32, tag="rden")
            nc.vector.reciprocal(out=rden, in_=denom)
            awbf = small.tile([1, S], bf16, tag="awbf")
            nc.vector.tensor_scalar_mul(awbf, escore, rden)
            nc.vector.tensor_scalar_mul(escore, escore, rden)
            nc.sync.dma_start(out=aw_out[b : b + 1, :], in_=escore)
            state[b]["awbf"] = awbf

        def load_nat(b):
            # natural-layout re-read for the context matmuls; only needed at
            # tail(b), i.e. after mm(b+1). Rides SWDGE behind the next batch's
            # casts so it never delays the transpose-critical SP ring.
            vnat = vn_pool.tile([P, SC, D], bf16, tag="vnat")
            r_inst = nc.gpsimd.dma_start(
                out=vnat,
                in_=state[b]["vdram"].rearrange("(sc sp) d -> sp sc d", sp=P),
            )
            state[b]["vnat"] = vnat

        def tail(b):
            vnat = state[b]["vnat"]
            awbf = state[b]["awbf"]
            # transpose aw [1, S] -> [s_part, sc] via K=1 matmuls against ones
            ps_awT = psawt.tile([P, SC], f32)
            for sc in range(SC):
                nc.tensor.matmul(
                    ps_awT[:, sc : sc + 1],
                    lhsT=awbf[0:1, sc * P : (sc + 1) * P],
                    rhs=ones1,
                    start=True,
                    stop=True,
                    skip_group_check=True,
                )
            awT = small.tile([P, SC], bf16, tag="awT")
            nc.vector.tensor_copy(out=awT, in_=ps_awT)

            # context[d] = sum_s aw[s] * values[s, d]
            ctxs = small.tile([1, D], f32, tag="ctxs", bufs=1)
            for h in range(2):
                ps_c = psctx.tile([1, 512], f32)
                for sc in range(SC):
                    nc.tensor.matmul(
                        ps_c,
                        lhsT=awT[:, sc : sc + 1],
                        rhs=vnat[:, sc, h * 512 : (h + 1) * 512],
                        start=(sc == 0),
                        stop=(sc == SC - 1),
                        skip_group_check=True,
                    )
                nc.vector.tensor_copy(out=ctxs[0:1, h * 512 : (h + 1) * 512], in_=ps_c)
            nc.sync.dma_start(out=ctx_out[b : b + 1, :], in_=ctxs)
            del state[b]

        setup_q(after_early=lambda: (load(0, blocks=[0]), load_w2()))
        load(0, blocks=[1, 2, 3])
        load(1)
        load_nat(0)
        mm(0)
        for b in range(1, BPC):
            load_nat(b)
            mm(b)
            tail(b - 1)
            if b + 1 < BPC:
                load(b + 1)
        tail(BPC - 1)


_NC_CACHE = None


def build_module():
    global _NC_CACHE
    if _NC_CACHE is not None:
        return _NC_CACHE
    nc = bacc.Bacc(
        "TRN2",
        target_bir_lowering=False,
        num_devices=N_CORES,
        num_swdge_queues=2,
    )
    io = {
        "vals": nc.dram_tensor("vals", (BPC, S, D), f32, kind="ExternalInput"),
        "qry": nc.dram_tensor("qry", (BPC, D), f32, kind="ExternalInput"),
        "w1": nc.dram_tensor("w1", (D, U), f32, kind="ExternalInput"),
        "b1": nc.dram_tensor("b1", (U,), f32, kind="ExternalInput"),
        "w2": nc.dram_tensor("w2", (D, U), f32, kind="ExternalInput"),
        "b2": nc.dram_tensor("b2", (U,), f32, kind="ExternalInput"),
        "wv": nc.dram_tensor("wv", (U, 1), f32, kind="ExternalInput"),
        "bv": nc.dram_tensor("bv", (1,), f32, kind="ExternalInput"),
        "ctx_out": nc.dram_tensor("ctx_out", (BPC, D), f32, kind="ExternalOutput"),
        "aw_out": nc.dram_tensor("aw_out", (BPC, S), f32, kind="ExternalOutput"),
    }
    with tile.TileContext(nc) as tc:
        emit(nc, tc, io)
    nc.finalize()
    _NC_CACHE = nc
    return nc


def make_in_maps(query, values, w1, b1, w2, b2, wv, bv):
    query = np.asarray(query, dtype=np.float32)
    values = np.asarray(values, dtype=np.float32)
    shared = {
        "w1": np.asarray(w1, dtype=np.float32),
        "b1": np.asarray(b1, dtype=np.float32),
        "w2": np.asarray(w2, dtype=np.float32),
        "b2": np.asarray(b2, dtype=np.float32),
        "wv": np.asarray(wv, dtype=np.float32),
        "bv": np.asarray(bv, dtype=np.float32),
    }
    in_maps = []
    for c in range(N_CORES):
        sl = slice(c * BPC, (c + 1) * BPC)
        in_maps.append(
            {
                "vals": np.ascontiguousarray(values[sl]),
                "qry": np.ascontiguousarray(query[sl]),
                **shared,
            }
        )
    return in_maps


def kernel(query, values, w1, b1, w2, b2, wv, bv):
    nc = build_module()
    in_maps = make_in_maps(query, values, w1, b1, w2, b2, wv, bv)
    res = run_bass_kernel_spmd(nc, in_maps, core_ids=list(range(N_CORES)))
    ctx = np.concatenate([res.results[c]["ctx_out"] for c in range(N_CORES)], axis=0)
    aw = np.concatenate([res.results[c]["aw_out"] for c in range(N_CORES)], axis=0)
    return ctx.astype(np.float32), aw[..., None].astype(np.float32)


# revision 69
# speedup vs baseline: 1.0188x; 1.0188x over previous
"""Bahdanau attention TRN2 Bass kernel.

Math (per batch b):
    q_proj = query @ w1 + b1                      # [U]
    v_proj = values[b] @ w2 + b2                  # [S, U]
    score  = tanh(q_proj + v_proj) @ wv + bv      # [S]
    aw     = softmax(score)                       # [S]
    ctx    = sum_s aw[s] * values[b, s, :]        # [D]

Sharding: batch B=32 split 4-per-core across 8 NeuronCores; weights replicated.

Per-core layout strategy (bf16 compute, fp32 accumulation):
  - values loaded fp32->bf16 (SWDGE cast DMA) into small staging tiles,
    bounced through a DRAM bf16 copy, re-read with DMA transpose into
    [d_part, s_free] tiles (vT) and re-read naturally into [s_part, d_free]
    tiles (vnat, used only by the context matmuls).
  - main matmul: psum[u(128), s(512)] += w2bf[d,u].T @ vT[d, s] over 8 d-chunks
  - tanh fused with bias add on ScalarE: bias = q_proj[u] + b1 + b2 per
    partition, broadcast along s.
  - score: psum[1, s] += wv[u].T @ tanh[u, s] accumulated over 8 u-chunks.
  - softmax on one partition ([1, 2048]); no max-subtraction needed (scores
    are bounded by |wv| * sqrt(U), far inside fp32 exp range).
  - aw transposed to [s_part, chunk] via 16 tiny K=1 matmuls against ones.
  - context: psum[1, d] += aw[s].T @ vnat[s, d] over 16 s-chunks.

The four batches are software-pipelined (mm(b), tail(b-1), load(b+1)) so each
batch's softmax latency chain and context matmuls overlap the next batch's
main matmuls; loads never hold long-lived SBUF slots so the DMA pipeline
never waits on compute.
"""

import numpy as np

import concourse.bass as bass
import concourse.mybir as mybir
import concourse.tile as tile
from concourse import bacc
from concourse.bass_utils import run_bass_kernel_spmd
from concourse.masks import make_identity

B, S, D, U = 32, 2048, 1024, 1024
N_CORES = 8
BPC = B // N_CORES  # batches per core
P = 128
DC = D // P  # 8 d-chunks
UC = U // P  # 8 u-chunks
SC = S // P  # 16 s-chunks
NSB = 4  # s-blocks per batch
SBS = S // NSB  # 512

f32 = mybir.dt.float32
bf16 = mybir.dt.bfloat16
AF = mybir.ActivationFunctionType
ALU = mybir.AluOpType
AX = mybir.AxisListType


def emit(nc: bass.Bass, tc: tile.TileContext, io: dict):
    vals, qry = io["vals"], io["qry"]
    w1, b1, w2, b2, wv, bv = io["w1"], io["b1"], io["w2"], io["b2"], io["wv"], io["bv"]
    ctx_out, aw_out = io["ctx_out"], io["aw_out"]

    with (
        tc.tile_pool(name="consts", bufs=1) as consts,
        tc.tile_pool(name="stage", bufs=2) as stage_pool,
        tc.tile_pool(name="vnat", bufs=2) as vn_pool,
        tc.tile_pool(name="vt", bufs=2) as vt_pool,
        tc.tile_pool(name="tanh", bufs=3) as tanh_pool,
        tc.tile_pool(name="small", bufs=2) as small,
        tc.tile_pool(name="dram", bufs=2, space="DRAM") as dram_pool,
        tc.tile_pool(name="psmm", bufs=2, space="PSUM") as psmm,
        tc.tile_pool(name="pssc", bufs=2, space="PSUM") as pssc,
        tc.tile_pool(name="psq", bufs=1, space="PSUM") as psq,
        tc.tile_pool(name="psawt", bufs=1, space="PSUM") as psawt,
        tc.tile_pool(name="psctx", bufs=2, space="PSUM") as psctx,
    ):
        # ---- small constants on the ACT HWDGE ring (SP ring and SWDGE stay
        # free for the values pipeline) ----
        w2bf = consts.tile([P, DC, U], bf16)
        wvbf = consts.tile([P, UC], bf16)
        wvf = consts.tile([P, UC], f32)
        b1s = consts.tile([P, UC], f32)
        b2s = consts.tile([P, UC], f32)
        bvs = consts.tile([1, 1], f32)
        ones1 = consts.tile([1, 1], bf16)

        def load_consts():
            with nc.allow_non_contiguous_dma(reason="tiny weight reshapes"):
                nc.scalar.dma_start(
                    out=wvf, in_=wv.rearrange("(uc up) o -> up (uc o)", up=P)
                )
                nc.scalar.dma_start(
                    out=b1s, in_=b1.rearrange("(uc up) -> up uc", up=P)
                )
                nc.scalar.dma_start(
                    out=b2s, in_=b2.rearrange("(uc up) -> up uc", up=P)
                )
                nc.scalar.dma_start(out=bvs, in_=bv.rearrange("(a o) -> a o", a=1))
            nc.vector.tensor_copy(out=wvbf, in_=wvf)
            nc.vector.memset(ones1, 1.0)

        def load_w2():
            nc.gpsimd.dma_start(
                out=w2bf, in_=w2.rearrange("(dc dp) u -> dp dc u", dp=P)
            )

        # ---- q_plus[u, b] = query @ w1 + b1 + b2 (tanh bias terms) ----
        # w1/query ride the ACT HWDGE ring as fp32 (cast on idle DVE) in
        # per-uc column blocks, so the q-proj matmuls start within ~2us and
        # never queue behind the SWDGE value-cast stream.
        q_plus = consts.tile([P, UC, BPC], f32)

        setup = {}

        def setup_q_pre(pool):
            # one contiguous 16KB load + PE transposes beats 8 strided
            # DMAs through the serialized DMA pipe at startup
            qnat = pool.tile([BPC, D], f32)
            nc.scalar.dma_start(out=qnat, in_=qry[:, :])
            id4 = pool.tile([BPC, BPC], f32)
            make_identity(nc, id4)
            # small consts go behind qnat so the first PE transpose isn't
            # stuck behind their slow strided descriptors
            load_consts()
            qTbf = pool.tile([P, DC, BPC], bf16)
            for dc in range(DC):
                ps_t = psq.tile([P, BPC], f32, tag="ps")
                nc.tensor.transpose(ps_t, qnat[:, dc * P : (dc + 1) * P], id4)
                nc.vector.tensor_copy(out=qTbf[:, dc, :], in_=ps_t)
            setup["pool"] = pool
            setup["qTbf"] = qTbf
            setup["w1_t"] = w1.rearrange("(dc dp) u -> dp dc u", dp=P)

        def q_step(uc):
            # one u-chunk of q_proj = query @ w1 + b1 + b2, interleaved into
            # mm(0)'s first s-block: q_plus[uc] is only read by tanh(uc),
            # which runs well after the main matmuls of u-chunk uc
            pool, qTbf, w1_t = setup["pool"], setup["qTbf"], setup["w1_t"]
            w1f = pool.tile([P, DC, P], f32, tag="w1f", bufs=2)
            nc.scalar.dma_start(out=w1f, in_=w1_t[:, :, uc * P : (uc + 1) * P])
            w1bf = pool.tile([P, DC, P], bf16, tag="w1bf", bufs=2)
            nc.vector.tensor_copy(out=w1bf, in_=w1f)
            ps = psq.tile([P, BPC], f32)
            for dc in range(DC):
                nc.tensor.matmul(
                    ps,
                    lhsT=w1bf[:, dc, :],
                    rhs=qTbf[:, dc, :],
                    start=(dc == 0),
                    stop=(dc == DC - 1),
                )
            nc.vector.tensor_scalar(
                out=q_plus[:, uc, :],
                in0=ps,
                scalar1=b1s[:, uc : uc + 1],
                scalar2=b2s[:, uc : uc + 1],
                op0=ALU.add,
                op1=ALU.add,
            )

        # ---- per-batch stages, software-pipelined ----
        # PE program order: mm(0), mm(1), tail(0), mm(2), tail(1), mm(3),
        # tail(2), tail(3) — so each batch's softmax (ACT/DVE latency chain)
        # overlaps the next batch's main matmuls instead of stalling PE.
        state = {}

        def load(b, blocks=None):
            # fp32 -> bf16 cast into a small staging tile (SWDGE), bounce via
            # DRAM, and DMA-transpose back into vT. Nothing here holds a
            # long-lived SBUF slot, so batch b+1's load never waits on batch
            # b-1's context matmuls.
            if b not in state:
                state[b] = {
                    "vT": vt_pool.tile([P, DC, S], bf16, tag="vT", name=f"vT{b}"),
                    "vdram": dram_pool.tile(
                        [S, D], bf16, tag="vdram", name=f"vdram{b}"
                    ),
                    "writes": {},
                }
            vT = state[b]["vT"]
            vbf_dram = state[b]["vdram"]
            vals_b = vals[b].rearrange("(sc sp) d -> sp sc d", sp=P)
            vdram_t = vbf_dram.rearrange("(sc sp) d -> sp sc d", sp=P)
            ncb = SC // NSB  # s-chunks per block
            for sb in blocks if blocks is not None else range(NSB):
                sl = slice(sb * ncb, (sb + 1) * ncb)
                stg = stage_pool.tile([P, ncb, D], bf16, tag="stg", name=f"stg{b}_{sb}")
                nc.gpsimd.dma_start(out=stg, in_=vals_b[:, sl, :])
                w_inst = nc.sync.dma_start(out=vdram_t[:, sl, :], in_=stg)
                state[b]["writes"][sb] = w_inst
                rows = slice(sb * SBS, (sb + 1) * SBS)
                for dc in range(DC):
                    t_inst = nc.sync.dma_start_transpose(
                        out=vT[:, dc, rows],
                        in_=vbf_dram[rows, dc * P : (dc + 1) * P],
                    )

        def mm(b):
            vT = state[b]["vT"]
            escore = small.tile([1, S], f32, tag="escore")
            for sb in range(NSB):
                ssl = slice(sb * SBS, (sb + 1) * SBS)
                ps_s = pssc.tile([1, SBS], f32)
                for uc in range(UC):
                    if b == 0 and sb == 0:
                        q_step(uc)
                    ps_v = psmm.tile([P, SBS], f32)
                    for dc in range(DC):
                        nc.tensor.matmul(
                            ps_v,
                            lhsT=w2bf[:, dc, uc * P : (uc + 1) * P],
                            rhs=vT[:, dc, ssl],
                            start=(dc == 0),
                            stop=(dc == DC - 1),
                        )
                    th = tanh_pool.tile([P, SBS], bf16)
                    nc.scalar.activation(
                        out=th,
                        in_=ps_v,
                        func=AF.Tanh,
                        bias=q_plus[:, uc, b : b + 1],
                        scale=1.0,
                    )
                    nc.tensor.matmul(
                        ps_s,
                        lhsT=wvbf[:, uc : uc + 1],
                        rhs=th,
                        start=(uc == 0),
                        stop=(uc == UC - 1),
                        skip_group_check=True,
                    )
                nc.scalar.activation(
                    out=escore[0:1, ssl],
                    in_=ps_s,
                    func=AF.Exp,
                    bias=bvs[0:1, :],
                    scale=1.0,
                )

            # softmax normalization on a single partition (in-place on escore)
            denom = small.tile([1, 1], f32, tag="denom")
            nc.vector.reduce_sum(out=denom, in_=escore, axis=AX.X)
            rden = small.tile([1, 1], f32, tag="rden")
            nc.vector.reciprocal(out=rden, in_=denom)
            awbf = small.tile([1, S], bf16, tag="awbf")
            nc.vector.tensor_scalar_mul(awbf, escore, rden)
            nc.vector.tensor_scalar_mul(escore, escore, rden)
            nc.sync.dma_start(out=aw_out[b : b + 1, :], in_=escore)
            state[b]["awbf"] = awbf

        def load_nat(b):
            # natural-layout re-read for the context matmuls; only needed at
            # tail(b), i.e. after mm(b+1). Rides SWDGE behind the next batch's
            # casts so it never delays the transpose-critical SP ring.
            vnat = vn_pool.tile([P, SC, D], bf16, tag="vnat")
            nc.sync.dma_start(
                out=vnat,
                in_=state[b]["vdram"].rearrange("(sc sp) d -> sp sc d", sp=P),
            )
            state[b]["vnat"] = vnat

        def tail(b):
            vnat = state[b]["vnat"]
            awbf = state[b]["awbf"]
            # transpose aw [1, S] -> [s_part, sc] via K=1 matmuls against ones
            ps_awT = psawt.tile([P, SC], f32)
            for sc in range(SC):
                nc.tensor.matmul(
                    ps_awT[:, sc : sc + 1],
                    lhsT=awbf[0:1, sc * P : (sc + 1) * P],
                    rhs=ones1,
                    start=True,
                    stop=True,
                    skip_group_check=True,
                )
            awT = small.tile([P, SC], bf16, tag="awT")
            nc.vector.tensor_copy(out=awT, in_=ps_awT)

            # context[d] = sum_s aw[s] * values[s, d]
            ctxs = small.tile([1, D], f32, tag="ctxs", bufs=1)
            for h in range(2):
                ps_c = psctx.tile([1, 512], f32)
                for sc in range(SC):
                    nc.tensor.matmul(
                        ps_c,
                        lhsT=awT[:, sc : sc + 1],
                        rhs=vnat[:, sc, h * 512 : (h + 1) * 512],
                        start=(sc == 0),
                        stop=(sc == SC - 1),
                        skip_group_check=True,
                    )
                nc.vector.tensor_copy(out=ctxs[0:1, h * 512 : (h + 1) * 512], in_=ps_c)
            nc.sync.dma_start(out=ctx_out[b : b + 1, :], in_=ctxs)
            del state[b]

        with tc.tile_pool(name="setup", bufs=1) as setup_pool:
            setup_q_pre(setup_pool)
            load(0, blocks=[0])
            load_w2()
            load(0, blocks=[1, 2, 3])
            load(1)
            load_nat(0)
            mm(0)
        for b in range(1, BPC):
            load_nat(b)
            mm(b)
            tail(b - 1)
            if b + 1 < BPC:
                load(b + 1)
        tail(BPC - 1)


_NC_CACHE = None


def build_module():
    global _NC_CACHE
    if _NC_CACHE is not None:
        return _NC_CACHE
    nc = bacc.Bacc(
        "TRN2",
        target_bir_lowering=False,
        num_devices=N_CORES,
        num_swdge_queues=2,
    )
    io = {
        "vals": nc.dram_tensor("vals", (BPC, S, D), f32, kind="ExternalInput"),
        "qry": nc.dram_tensor("qry", (BPC, D), f32, kind="ExternalInput"),
        "w1": nc.dram_tensor("w1", (D, U), f32, kind="ExternalInput"),
        "b1": nc.dram_tensor("b1", (U,), f32, kind="ExternalInput"),
        "w2": nc.dram_tensor("w2", (D, U), f32, kind="ExternalInput"),
        "b2": nc.dram_tensor("b2", (U,), f32, kind="ExternalInput"),
        "wv": nc.dram_tensor("wv", (U, 1), f32, kind="ExternalInput"),
        "bv": nc.dram_tensor("bv", (1,), f32, kind="ExternalInput"),
        "ctx_out": nc.dram_tensor("ctx_out", (BPC, D), f32, kind="ExternalOutput"),
        "aw_out": nc.dram_tensor("aw_out", (BPC, S), f32, kind="ExternalOutput"),
    }
    with tile.TileContext(nc) as tc:
        emit(nc, tc, io)
    nc.finalize()
    _NC_CACHE = nc
    return nc


def make_in_maps(query, values, w1, b1, w2, b2, wv, bv):
    query = np.asarray(query, dtype=np.float32)
    values = np.asarray(values, dtype=np.float32)
    shared = {
        "w1": np.asarray(w1, dtype=np.float32),
        "b1": np.asarray(b1, dtype=np.float32),
        "w2": np.asarray(w2, dtype=np.float32),
        "b2": np.asarray(b2, dtype=np.float32),
        "wv": np.asarray(wv, dtype=np.float32),
        "bv": np.asarray(bv, dtype=np.float32),
    }
    in_maps = []
    for c in range(N_CORES):
        sl = slice(c * BPC, (c + 1) * BPC)
        in_maps.append(
            {
                "vals": np.ascontiguousarray(values[sl]),
                "qry": np.ascontiguousarray(query[sl]),
                **shared,
            }
        )
    return in_maps


def kernel(query, values, w1, b1, w2, b2, wv, bv):
    nc = build_module()
    in_maps = make_in_maps(query, values, w1, b1, w2, b2, wv, bv)
    res = run_bass_kernel_spmd(nc, in_maps, core_ids=list(range(N_CORES)))
    ctx = np.concatenate([res.results[c]["ctx_out"] for c in range(N_CORES)], axis=0)
    aw = np.concatenate([res.results[c]["aw_out"] for c in range(N_CORES)], axis=0)
    return ctx.astype(np.float32), aw[..., None].astype(np.float32)


# revision 87
# speedup vs baseline: 1.0512x; 1.0318x over previous
"""Bahdanau attention TRN2 Bass kernel.

Math (per batch b):
    q_proj = query @ w1 + b1                      # [U]
    v_proj = values[b] @ w2 + b2                  # [S, U]
    score  = tanh(q_proj + v_proj) @ wv + bv      # [S]
    aw     = softmax(score)                       # [S]
    ctx    = sum_s aw[s] * values[b, s, :]        # [D]

Sharding: batch B=32 split 4-per-core across 8 NeuronCores; weights replicated.

Per-core layout strategy (bf16 compute, fp32 accumulation):
  - values loaded fp32->bf16 (SWDGE cast DMA) into small staging tiles,
    bounced through a DRAM bf16 copy, re-read with DMA transpose into
    [d_part, s_free] tiles (vT) and re-read naturally into [s_part, d_free]
    tiles (vnat, used only by the context matmuls).
  - main matmul: psum[u(128), s(512)] += w2bf[d,u].T @ vT[d, s] over 8 d-chunks
  - tanh fused with bias add on ScalarE: bias = q_proj[u] + b1 + b2 per
    partition, broadcast along s.
  - score: psum[1, s] += wv[u].T @ tanh[u, s] accumulated over 8 u-chunks.
  - softmax on one partition ([1, 2048]); no max-subtraction needed (scores
    are bounded by |wv| * sqrt(U), far inside fp32 exp range).
  - aw transposed to [s_part, chunk] via 16 tiny K=1 matmuls against ones.
  - context: psum[1, d] += aw[s].T @ vnat[s, d] over 16 s-chunks.

The four batches are software-pipelined (mm(b), tail(b-1), load(b+1)) so each
batch's softmax latency chain and context matmuls overlap the next batch's
main matmuls; loads never hold long-lived SBUF slots so the DMA pipeline
never waits on compute.
"""

import numpy as np

import concourse.bass as bass
import concourse.mybir as mybir
import concourse.tile as tile
from concourse import bacc
from concourse.bass_utils import run_bass_kernel_spmd
from concourse.masks import make_identity

B, S, D, U = 32, 2048, 1024, 1024
N_CORES = 8
BPC = B // N_CORES  # batches per core
P = 128
DC = D // P  # 8 d-chunks
UC = U // P  # 8 u-chunks
SC = S // P  # 16 s-chunks
NSB = 4  # s-blocks per batch
SBS = S // NSB  # 512

f32 = mybir.dt.float32
bf16 = mybir.dt.bfloat16
AF = mybir.ActivationFunctionType
ALU = mybir.AluOpType
AX = mybir.AxisListType


def emit(nc: bass.Bass, tc: tile.TileContext, io: dict):
    vals, qry = io["vals"], io["qry"]
    w1, b1, w2, b2, wv, bv = io["w1"], io["b1"], io["w2"], io["b2"], io["wv"], io["bv"]
    ctx_out, aw_out = io["ctx_out"], io["aw_out"]

    with (
        tc.tile_pool(name="consts", bufs=1) as consts,
        tc.tile_pool(name="stage", bufs=2) as stage_pool,
        tc.tile_pool(name="vnat", bufs=2) as vn_pool,
        tc.tile_pool(name="vt", bufs=2) as vt_pool,
        tc.tile_pool(name="tanh", bufs=3) as tanh_pool,
        tc.tile_pool(name="small", bufs=2) as small,
        tc.tile_pool(name="dram", bufs=2, space="DRAM") as dram_pool,
        tc.tile_pool(name="psmm", bufs=2, space="PSUM") as psmm,
        tc.tile_pool(name="pssc", bufs=2, space="PSUM") as pssc,
        tc.tile_pool(name="psq", bufs=1, space="PSUM") as psq,
        tc.tile_pool(name="psawt", bufs=1, space="PSUM") as psawt,
        tc.tile_pool(name="psctx", bufs=2, space="PSUM") as psctx,
    ):
        # ---- small constants on the ACT HWDGE ring (SP ring and SWDGE stay
        # free for the values pipeline) ----
        w2bf = consts.tile([P, DC, U], bf16)
        wvbf = consts.tile([P, UC], bf16)
        wvf = consts.tile([P, UC], f32)
        b1s = consts.tile([P, UC], f32)
        b2s = consts.tile([P, UC], f32)
        bvs = consts.tile([1, 1], f32)
        ones1 = consts.tile([1, 1], bf16)
        id128 = consts.tile([P, P], bf16)

        def load_consts():
            with nc.allow_non_contiguous_dma(reason="tiny weight reshapes"):
                nc.scalar.dma_start(
                    out=wvf, in_=wv.rearrange("(uc up) o -> up (uc o)", up=P)
                )
                nc.scalar.dma_start(
                    out=b1s, in_=b1.rearrange("(uc up) -> up uc", up=P)
                )
                nc.scalar.dma_start(
                    out=b2s, in_=b2.rearrange("(uc up) -> up uc", up=P)
                )
                nc.scalar.dma_start(out=bvs, in_=bv.rearrange("(a o) -> a o", a=1))
            nc.vector.tensor_copy(out=wvbf, in_=wvf)
            nc.vector.memset(ones1, 1.0)
            make_identity(nc, id128)

        def load_w2(half=None):
            w2_t = w2.rearrange("(dc dp) u -> dp dc u", dp=P)
            if half is None:
                nc.gpsimd.dma_start(out=w2bf, in_=w2_t)
            else:
                hd = slice(half * DC // 2, (half + 1) * DC // 2)
                nc.gpsimd.dma_start(out=w2bf[:, hd, :], in_=w2_t[:, hd, :])

        # ---- q_plus[u, b] = query @ w1 + b1 + b2 (tanh bias terms) ----
        # w1/query ride the ACT HWDGE ring as fp32 (cast on idle DVE) in
        # per-uc column blocks, so the q-proj matmuls start within ~2us and
        # never queue behind the SWDGE value-cast stream.
        q_plus = consts.tile([P, UC, BPC], f32)

        setup = {}

        def setup_q_pre(pool):
            # one contiguous 16KB load + PE transposes beats 8 strided
            # DMAs through the serialized DMA pipe at startup
            qnat = pool.tile([BPC, D], f32)
            nc.scalar.dma_start(out=qnat, in_=qry[:, :])
            id4 = pool.tile([BPC, BPC], f32)
            make_identity(nc, id4)
            # small consts go behind qnat so the first PE transpose isn't
            # stuck behind their slow strided descriptors
            load_consts()
            qTbf = pool.tile([P, DC, BPC], bf16)
            for dc in range(DC):
                ps_t = psq.tile([P, BPC], f32, tag="ps")
                nc.tensor.transpose(ps_t, qnat[:, dc * P : (dc + 1) * P], id4)
                nc.vector.tensor_copy(out=qTbf[:, dc, :], in_=ps_t)
            setup["pool"] = pool
            setup["qTbf"] = qTbf
            setup["w1_t"] = w1.rearrange("(dc dp) u -> dp dc u", dp=P)

        def q_step(uc):
            # one u-chunk of q_proj = query @ w1 + b1 + b2, interleaved into
            # mm(0)'s first s-block: q_plus[uc] is only read by tanh(uc),
            # which runs well after the main matmuls of u-chunk uc
            pool, qTbf, w1_t = setup["pool"], setup["qTbf"], setup["w1_t"]
            w1f = pool.tile([P, DC, P], f32, tag="w1f", bufs=2)
            nc.scalar.dma_start(out=w1f, in_=w1_t[:, :, uc * P : (uc + 1) * P])
            w1bf = pool.tile([P, DC, P], bf16, tag="w1bf", bufs=2)
            nc.vector.tensor_copy(out=w1bf, in_=w1f)
            ps = psq.tile([P, BPC], f32)
            for dc in range(DC):
                nc.tensor.matmul(
                    ps,
                    lhsT=w1bf[:, dc, :],
                    rhs=qTbf[:, dc, :],
                    start=(dc == 0),
                    stop=(dc == DC - 1),
                )
            nc.vector.tensor_scalar(
                out=q_plus[:, uc, :],
                in0=ps,
                scalar1=b1s[:, uc : uc + 1],
                scalar2=b2s[:, uc : uc + 1],
                op0=ALU.add,
                op1=ALU.add,
            )

        # ---- per-batch stages, software-pipelined ----
        # PE program order: mm(0), mm(1), tail(0), mm(2), tail(1), mm(3),
        # tail(2), tail(3) — so each batch's softmax (ACT/DVE latency chain)
        # overlaps the next batch's main matmuls instead of stalling PE.
        state = {}

        def load(b, blocks=None):
            # fp32 -> bf16 cast into a small staging tile (SWDGE), bounce via
            # DRAM, and DMA-transpose back into vT. Nothing here holds a
            # long-lived SBUF slot, so batch b+1's load never waits on batch
            # b-1's context matmuls.
            if b not in state:
                state[b] = {
                    "vT": vt_pool.tile([P, DC, S], bf16, tag="vT", name=f"vT{b}"),
                    "vdram": dram_pool.tile(
                        [S, D], bf16, tag="vdram", name=f"vdram{b}"
                    ),
                    "writes": {},
                }
            vT = state[b]["vT"]
            vbf_dram = state[b]["vdram"]
            vals_b = vals[b].rearrange("(sc sp) d -> sp sc d", sp=P)
            vdram_t = vbf_dram.rearrange("(sc sp) d -> sp sc d", sp=P)
            ncb = SC // NSB  # s-chunks per block
            for sb in blocks if blocks is not None else range(NSB):
                sl = slice(sb * ncb, (sb + 1) * ncb)
                stg = stage_pool.tile([P, ncb, D], bf16, tag="stg", name=f"stg{b}_{sb}")
                nc.gpsimd.dma_start(out=stg, in_=vals_b[:, sl, :])
                w_inst = nc.sync.dma_start(out=vdram_t[:, sl, :], in_=stg)
                state[b]["writes"][sb] = w_inst
                if b == 0 and sb <= 2:
                    # startup-critical blocks: PE is idle here anyway, so
                    # transpose them on the PE directly from the staging tile
                    # (borrowing the main-matmul PSUM slots) instead of
                    # waiting for the DRAM round-trip
                    for sc in range(ncb):
                        for dc in range(DC):
                            ps_t = psmm.tile([P, P], bf16, tag="ps_v", name="ps_tp")
                            nc.tensor.transpose(
                                ps_t, stg[:, sc, dc * P : (dc + 1) * P], id128
                            )
                            nc.vector.tensor_copy(
                                out=vT[:, dc, (sb * ncb + sc) * P : (sb * ncb + sc + 1) * P],
                                in_=ps_t,
                            )
                    continue
                rows = slice(sb * SBS, (sb + 1) * SBS)
                for dc in range(DC):
                    t_inst = nc.sync.dma_start_transpose(
                        out=vT[:, dc, rows],
                        in_=vbf_dram[rows, dc * P : (dc + 1) * P],
                    )

        def mm(b):
            vT = state[b]["vT"]
            escore = small.tile([1, S], f32, tag="escore")
            for sb in range(NSB):
                ssl = slice(sb * SBS, (sb + 1) * SBS)
                ps_s = pssc.tile([1, SBS], f32)
                for uc in range(UC):
                    if b == 0 and sb == 0:
                        q_step(uc)
                    ps_v = psmm.tile([P, SBS], f32)
                    for dc in range(DC):
                        nc.tensor.matmul(
                            ps_v,
                            lhsT=w2bf[:, dc, uc * P : (uc + 1) * P],
                            rhs=vT[:, dc, ssl],
                            start=(dc == 0),
                            stop=(dc == DC - 1),
                        )
                    th = tanh_pool.tile([P, SBS], bf16)
                    nc.scalar.activation(
                        out=th,
                        in_=ps_v,
                        func=AF.Tanh,
                        bias=q_plus[:, uc, b : b + 1],
                        scale=1.0,
                    )
                    nc.tensor.matmul(
                        ps_s,
                        lhsT=wvbf[:, uc : uc + 1],
                        rhs=th,
                        start=(uc == 0),
                        stop=(uc == UC - 1),
                        skip_group_check=True,
                    )
                nc.scalar.activation(
                    out=escore[0:1, ssl],
                    in_=ps_s,
                    func=AF.Exp,
                    bias=bvs[0:1, :],
                    scale=1.0,
                )

            # softmax normalization on a single partition (in-place on escore)
            denom = small.tile([1, 1], f32, tag="denom")
            nc.vector.reduce_sum(out=denom, in_=escore, axis=AX.X)
            rden = small.tile([1, 1], f32, tag="rden")
            nc.vector.reciprocal(out=rden, in_=denom)
            awbf = small.tile([1, S], bf16, tag="awbf")
            nc.vector.tensor_scalar_mul(awbf, escore, rden)
            nc.vector.tensor_scalar_mul(escore, escore, rden)
            nc.sync.dma_start(out=aw_out[b : b + 1, :], in_=escore)
            state[b]["awbf"] = awbf

        def load_nat(b):
            # natural-layout re-read for the context matmuls; only needed at
            # tail(b), i.e. after mm(b+1). Rides SWDGE behind the next batch's
            # casts so it never delays the transpose-critical SP ring.
            vnat = vn_pool.tile([P, SC, D], bf16, tag="vnat")
            nc.sync.dma_start(
                out=vnat,
                in_=state[b]["vdram"].rearrange("(sc sp) d -> sp sc d", sp=P),
            )
            state[b]["vnat"] = vnat

        def tail(b):
            vnat = state[b]["vnat"]
            awbf = state[b]["awbf"]
            # transpose aw [1, S] -> [s_part, sc] via K=1 matmuls against ones
            ps_awT = psawt.tile([P, SC], f32)
            for sc in range(SC):
                nc.tensor.matmul(
                    ps_awT[:, sc : sc + 1],
                    lhsT=awbf[0:1, sc * P : (sc + 1) * P],
                    rhs=ones1,
                    start=True,
                    stop=True,
                    skip_group_check=True,
                )
            awT = small.tile([P, SC], bf16, tag="awT")
            nc.vector.tensor_copy(out=awT, in_=ps_awT)

            # context[d] = sum_s aw[s] * values[s, d]
            ctxs = small.tile([1, D], f32, tag="ctxs", bufs=1)
            for h in range(2):
                ps_c = psctx.tile([1, 512], f32)
                for sc in range(SC):
                    nc.tensor.matmul(
                        ps_c,
                        lhsT=awT[:, sc : sc + 1],
                        rhs=vnat[:, sc, h * 512 : (h + 1) * 512],
                        start=(sc == 0),
                        stop=(sc == SC - 1),
                        skip_group_check=True,
                    )
                nc.vector.tensor_copy(out=ctxs[0:1, h * 512 : (h + 1) * 512], in_=ps_c)
            nc.sync.dma_start(out=ctx_out[b : b + 1, :], in_=ctxs)
            del state[b]

        with tc.tile_pool(name="setup", bufs=1) as setup_pool:
            setup_q_pre(setup_pool)
            load_w2(half=0)
            load(0, blocks=[0])
            load_w2(half=1)
            load(0, blocks=[1, 2, 3])
            load(1)
            load_nat(0)
            mm(0)
        for b in range(1, BPC):
            load_nat(b)
            mm(b)
            tail(b - 1)
            if b + 1 < BPC:
                load(b + 1)
        tail(BPC - 1)


_NC_CACHE = None


def build_module():
    global _NC_CACHE
    if _NC_CACHE is not None:
        return _NC_CACHE
    nc = bacc.Bacc(
        "TRN2",
        target_bir_lowering=False,
        num_devices=N_CORES,
        num_swdge_queues=2,
    )
    io = {
        "vals": nc.dram_tensor("vals", (BPC, S, D), f32, kind="ExternalInput"),
        "qry": nc.dram_tensor("qry", (BPC, D), f32, kind="ExternalInput"),
        "w1": nc.dram_tensor("w1", (D, U), f32, kind="ExternalInput"),
        "b1": nc.dram_tensor("b1", (U,), f32, kind="ExternalInput"),
        "w2": nc.dram_tensor("w2", (D, U), f32, kind="ExternalInput"),
        "b2": nc.dram_tensor("b2", (U,), f32, kind="ExternalInput"),
        "wv": nc.dram_tensor("wv", (U, 1), f32, kind="ExternalInput"),
        "bv": nc.dram_tensor("bv", (1,), f32, kind="ExternalInput"),
        "ctx_out": nc.dram_tensor("ctx_out", (BPC, D), f32, kind="ExternalOutput"),
        "aw_out": nc.dram_tensor("aw_out", (BPC, S), f32, kind="ExternalOutput"),
    }
    with tile.TileContext(nc) as tc:
        emit(nc, tc, io)
    nc.finalize()
    _NC_CACHE = nc
    return nc


def make_in_maps(query, values, w1, b1, w2, b2, wv, bv):
    query = np.asarray(query, dtype=np.float32)
    values = np.asarray(values, dtype=np.float32)
    shared = {
        "w1": np.asarray(w1, dtype=np.float32),
        "b1": np.asarray(b1, dtype=np.float32),
        "w2": np.asarray(w2, dtype=np.float32),
        "b2": np.asarray(b2, dtype=np.float32),
        "wv": np.asarray(wv, dtype=np.float32),
        "bv": np.asarray(bv, dtype=np.float32),
    }
    in_maps = []
    for c in range(N_CORES):
        sl = slice(c * BPC, (c + 1) * BPC)
        in_maps.append(
            {
                "vals": np.ascontiguousarray(values[sl]),
                "qry": np.ascontiguousarray(query[sl]),
                **shared,
            }
        )
    return in_maps


def kernel(query, values, w1, b1, w2, b2, wv, bv):
    nc = build_module()
    in_maps = make_in_maps(query, values, w1, b1, w2, b2, wv, bv)
    res = run_bass_kernel_spmd(nc, in_maps, core_ids=list(range(N_CORES)))
    ctx = np.concatenate([res.results[c]["ctx_out"] for c in range(N_CORES)], axis=0)
    aw = np.concatenate([res.results[c]["aw_out"] for c in range(N_CORES)], axis=0)
    return ctx.astype(np.float32), aw[..., None].astype(np.float32)


# revision 88
# speedup vs baseline: 1.0666x; 1.0147x over previous
"""Bahdanau attention TRN2 Bass kernel.

Math (per batch b):
    q_proj = query @ w1 + b1                      # [U]
    v_proj = values[b] @ w2 + b2                  # [S, U]
    score  = tanh(q_proj + v_proj) @ wv + bv      # [S]
    aw     = softmax(score)                       # [S]
    ctx    = sum_s aw[s] * values[b, s, :]        # [D]

Sharding: batch B=32 split 4-per-core across 8 NeuronCores; weights replicated.

Per-core layout strategy (bf16 compute, fp32 accumulation):
  - values loaded fp32->bf16 (SWDGE cast DMA) into small staging tiles,
    bounced through a DRAM bf16 copy, re-read with DMA transpose into
    [d_part, s_free] tiles (vT) and re-read naturally into [s_part, d_free]
    tiles (vnat, used only by the context matmuls).
  - main matmul: psum[u(128), s(512)] += w2bf[d,u].T @ vT[d, s] over 8 d-chunks
  - tanh fused with bias add on ScalarE: bias = q_proj[u] + b1 + b2 per
    partition, broadcast along s.
  - score: psum[1, s] += wv[u].T @ tanh[u, s] accumulated over 8 u-chunks.
  - softmax on one partition ([1, 2048]); no max-subtraction needed (scores
    are bounded by |wv| * sqrt(U), far inside fp32 exp range).
  - aw transposed to [s_part, chunk] via 16 tiny K=1 matmuls against ones.
  - context: psum[1, d] += aw[s].T @ vnat[s, d] over 16 s-chunks.

The four batches are software-pipelined (mm(b), tail(b-1), load(b+1)) so each
batch's softmax latency chain and context matmuls overlap the next batch's
main matmuls; loads never hold long-lived SBUF slots so the DMA pipeline
never waits on compute.
"""

import numpy as np

import concourse.bass as bass
import concourse.mybir as mybir
import concourse.tile as tile
from concourse import bacc
from concourse.bass_utils import run_bass_kernel_spmd
from concourse.masks import make_identity

B, S, D, U = 32, 2048, 1024, 1024
N_CORES = 8
BPC = B // N_CORES  # batches per core
P = 128
DC = D // P  # 8 d-chunks
UC = U // P  # 8 u-chunks
SC = S // P  # 16 s-chunks
NSB = 4  # s-blocks per batch
SBS = S // NSB  # 512

f32 = mybir.dt.float32
bf16 = mybir.dt.bfloat16
AF = mybir.ActivationFunctionType
ALU = mybir.AluOpType
AX = mybir.AxisListType


def emit(nc: bass.Bass, tc: tile.TileContext, io: dict):
    vals, qry = io["vals"], io["qry"]
    w1, b1, w2, b2, wv, bv = io["w1"], io["b1"], io["w2"], io["b2"], io["wv"], io["bv"]
    ctx_out, aw_out = io["ctx_out"], io["aw_out"]

    with (
        tc.tile_pool(name="consts", bufs=1) as consts,
        tc.tile_pool(name="stage", bufs=2) as stage_pool,
        tc.tile_pool(name="vnat", bufs=2) as vn_pool,
        tc.tile_pool(name="vt", bufs=2) as vt_pool,
        tc.tile_pool(name="tanh", bufs=3) as tanh_pool,
        tc.tile_pool(name="small", bufs=2) as small,
        tc.tile_pool(name="dram", bufs=2, space="DRAM") as dram_pool,
        tc.tile_pool(name="psmm", bufs=3, space="PSUM") as psmm,
        tc.tile_pool(name="pssc", bufs=2, space="PSUM") as pssc,
        tc.tile_pool(name="psq", bufs=1, space="PSUM") as psq,
        tc.tile_pool(name="psctx", bufs=2, space="PSUM") as psctx,
    ):
        # ---- small constants on the ACT HWDGE ring (SP ring and SWDGE stay
        # free for the values pipeline) ----
        w2bf = consts.tile([P, DC, U], bf16)
        wvbf = consts.tile([P, UC], bf16)
        wvf = consts.tile([P, UC], f32)
        b1s = consts.tile([P, UC], f32)
        b2s = consts.tile([P, UC], f32)
        bvs = consts.tile([1, 1], f32)
        ones1 = consts.tile([1, 1], bf16)
        id128 = consts.tile([P, P], bf16)

        def load_consts():
            with nc.allow_non_contiguous_dma(reason="tiny weight reshapes"):
                nc.scalar.dma_start(
                    out=wvf, in_=wv.rearrange("(uc up) o -> up (uc o)", up=P)
                )
                nc.scalar.dma_start(
                    out=b1s, in_=b1.rearrange("(uc up) -> up uc", up=P)
                )
                nc.scalar.dma_start(
                    out=b2s, in_=b2.rearrange("(uc up) -> up uc", up=P)
                )
                nc.scalar.dma_start(out=bvs, in_=bv.rearrange("(a o) -> a o", a=1))
            nc.vector.tensor_copy(out=wvbf, in_=wvf)
            nc.vector.memset(ones1, 1.0)
            make_identity(nc, id128)

        def load_w2(half=None):
            w2_t = w2.rearrange("(dc dp) u -> dp dc u", dp=P)
            if half is None:
                nc.gpsimd.dma_start(out=w2bf, in_=w2_t)
            else:
                hd = slice(half * DC // 2, (half + 1) * DC // 2)
                nc.gpsimd.dma_start(out=w2bf[:, hd, :], in_=w2_t[:, hd, :])

        # ---- q_plus[u, b] = query @ w1 + b1 + b2 (tanh bias terms) ----
        # w1/query ride the ACT HWDGE ring as fp32 (cast on idle DVE) in
        # per-uc column blocks, so the q-proj matmuls start within ~2us and
        # never queue behind the SWDGE value-cast stream.
        q_plus = consts.tile([P, UC, BPC], f32)

        setup = {}

        def setup_q_pre(pool):
            # one contiguous 16KB load + PE transposes beats 8 strided
            # DMAs through the serialized DMA pipe at startup
            qnat = pool.tile([BPC, D], f32)
            nc.scalar.dma_start(out=qnat, in_=qry[:, :])
            id4 = pool.tile([BPC, BPC], f32)
            make_identity(nc, id4)
            # small consts go behind qnat so the first PE transpose isn't
            # stuck behind their slow strided descriptors
            load_consts()
            qTbf = pool.tile([P, DC, BPC], bf16)
            for dc in range(DC):
                ps_t = psq.tile([P, BPC], f32, tag="ps")
                nc.tensor.transpose(ps_t, qnat[:, dc * P : (dc + 1) * P], id4)
                nc.vector.tensor_copy(out=qTbf[:, dc, :], in_=ps_t)
            setup["pool"] = pool
            setup["qTbf"] = qTbf
            setup["w1_t"] = w1.rearrange("(dc dp) u -> dp dc u", dp=P)

        def q_step(uc):
            # one u-chunk of q_proj = query @ w1 + b1 + b2, interleaved into
            # mm(0)'s first s-block: q_plus[uc] is only read by tanh(uc),
            # which runs well after the main matmuls of u-chunk uc
            pool, qTbf, w1_t = setup["pool"], setup["qTbf"], setup["w1_t"]
            w1f = pool.tile([P, DC, P], f32, tag="w1f", bufs=2)
            nc.scalar.dma_start(out=w1f, in_=w1_t[:, :, uc * P : (uc + 1) * P])
            w1bf = pool.tile([P, DC, P], bf16, tag="w1bf", bufs=2)
            nc.vector.tensor_copy(out=w1bf, in_=w1f)
            ps = psq.tile([P, BPC], f32)
            for dc in range(DC):
                nc.tensor.matmul(
                    ps,
                    lhsT=w1bf[:, dc, :],
                    rhs=qTbf[:, dc, :],
                    start=(dc == 0),
                    stop=(dc == DC - 1),
                )
            nc.vector.tensor_scalar(
                out=q_plus[:, uc, :],
                in0=ps,
                scalar1=b1s[:, uc : uc + 1],
                scalar2=b2s[:, uc : uc + 1],
                op0=ALU.add,
                op1=ALU.add,
            )

        # ---- per-batch stages, software-pipelined ----
        # PE program order: mm(0), mm(1), tail(0), mm(2), tail(1), mm(3),
        # tail(2), tail(3) — so each batch's softmax (ACT/DVE latency chain)
        # overlaps the next batch's main matmuls instead of stalling PE.
        state = {}

        def load(b, blocks=None):
            # fp32 -> bf16 cast into a small staging tile (SWDGE), bounce via
            # DRAM, and DMA-transpose back into vT. Nothing here holds a
            # long-lived SBUF slot, so batch b+1's load never waits on batch
            # b-1's context matmuls.
            if b not in state:
                state[b] = {
                    "vT": vt_pool.tile([P, DC, S], bf16, tag="vT", name=f"vT{b}"),
                    "vdram": dram_pool.tile(
                        [S, D], bf16, tag="vdram", name=f"vdram{b}"
                    ),
                    "writes": {},
                }
            vT = state[b]["vT"]
            vbf_dram = state[b]["vdram"]
            vals_b = vals[b].rearrange("(sc sp) d -> sp sc d", sp=P)
            vdram_t = vbf_dram.rearrange("(sc sp) d -> sp sc d", sp=P)
            ncb = SC // NSB  # s-chunks per block
            for sb in blocks if blocks is not None else range(NSB):
                sl = slice(sb * ncb, (sb + 1) * ncb)
                stg = stage_pool.tile([P, ncb, D], bf16, tag="stg", name=f"stg{b}_{sb}")
                nc.gpsimd.dma_start(out=stg, in_=vals_b[:, sl, :])
                w_inst = nc.sync.dma_start(out=vdram_t[:, sl, :], in_=stg)
                state[b]["writes"][sb] = w_inst
                if b == 0 and sb <= 2:
                    # startup-critical blocks: PE is idle here anyway, so
                    # transpose them on the PE directly from the staging tile
                    # (borrowing the main-matmul PSUM slots) instead of
                    # waiting for the DRAM round-trip
                    for sc in range(ncb):
                        for dc in range(DC):
                            ps_t = psmm.tile([P, P], bf16, tag="ps_v", name="ps_tp")
                            nc.tensor.transpose(
                                ps_t, stg[:, sc, dc * P : (dc + 1) * P], id128
                            )
                            nc.vector.tensor_copy(
                                out=vT[:, dc, (sb * ncb + sc) * P : (sb * ncb + sc + 1) * P],
                                in_=ps_t,
                            )
                    continue
                rows = slice(sb * SBS, (sb + 1) * SBS)
                for dc in range(DC):
                    t_inst = nc.sync.dma_start_transpose(
                        out=vT[:, dc, rows],
                        in_=vbf_dram[rows, dc * P : (dc + 1) * P],
                    )

        def mm(b):
            vT = state[b]["vT"]
            escore = small.tile([1, S], f32, tag="escore")
            for sb in range(NSB):
                ssl = slice(sb * SBS, (sb + 1) * SBS)
                ps_s = pssc.tile([1, SBS], f32)
                for uc in range(UC):
                    if b == 0 and sb == 0:
                        q_step(uc)
                    ps_v = psmm.tile([P, SBS], f32)
                    for dc in range(DC):
                        nc.tensor.matmul(
                            ps_v,
                            lhsT=w2bf[:, dc, uc * P : (uc + 1) * P],
                            rhs=vT[:, dc, ssl],
                            start=(dc == 0),
                            stop=(dc == DC - 1),
                        )
                    th = tanh_pool.tile([P, SBS], bf16)
                    nc.scalar.activation(
                        out=th,
                        in_=ps_v,
                        func=AF.Tanh,
                        bias=q_plus[:, uc, b : b + 1],
                        scale=1.0,
                    )
                    nc.tensor.matmul(
                        ps_s,
                        lhsT=wvbf[:, uc : uc + 1],
                        rhs=th,
                        start=(uc == 0),
                        stop=(uc == UC - 1),
                        skip_group_check=True,
                    )
                nc.scalar.activation(
                    out=escore[0:1, ssl],
                    in_=ps_s,
                    func=AF.Exp,
                    bias=bvs[0:1, :],
                    scale=1.0,
                )

            # softmax normalization on a single partition (in-place on escore)
            denom = small.tile([1, 1], f32, tag="denom")
            nc.vector.reduce_sum(out=denom, in_=escore, axis=AX.X)
            rden = small.tile([1, 1], f32, tag="rden")
            nc.vector.reciprocal(out=rden, in_=denom)
            awbf = small.tile([1, S], bf16, tag="awbf")
            nc.vector.tensor_scalar_mul(awbf, escore, rden)
            nc.vector.tensor_scalar_mul(escore, escore, rden)
            nc.sync.dma_start(out=aw_out[b : b + 1, :], in_=escore)
            state[b]["awbf"] = awbf

        def load_nat(b):
            # natural-layout re-read for the context matmuls; only needed at
            # tail(b), i.e. after mm(b+1). Rides SWDGE behind the next batch's
            # casts so it never delays the transpose-critical SP ring.
            vnat = vn_pool.tile([P, SC, D], bf16, tag="vnat")
            nc.sync.dma_start(
                out=vnat,
                in_=state[b]["vdram"].rearrange("(sc sp) d -> sp sc d", sp=P),
            )
            state[b]["vnat"] = vnat

        def tail(b):
            vnat = state[b]["vnat"]
            awbf = state[b]["awbf"]
            # transpose aw [1, S] -> [s_part, sc] via K=1 matmuls against ones
            ps_awT = psq.tile([P, SC], f32, tag="ps")
            for sc in range(SC):
                nc.tensor.matmul(
                    ps_awT[:, sc : sc + 1],
                    lhsT=awbf[0:1, sc * P : (sc + 1) * P],
                    rhs=ones1,
                    start=True,
                    stop=True,
                    skip_group_check=True,
                )
            awT = small.tile([P, SC], bf16, tag="awT")
            nc.vector.tensor_copy(out=awT, in_=ps_awT)

            # context[d] = sum_s aw[s] * values[s, d]
            ctxs = small.tile([1, D], f32, tag="ctxs", bufs=1)
            for h in range(2):
                ps_c = psctx.tile([1, 512], f32)
                for sc in range(SC):
                    nc.tensor.matmul(
                        ps_c,
                        lhsT=awT[:, sc : sc + 1],
                        rhs=vnat[:, sc, h * 512 : (h + 1) * 512],
                        start=(sc == 0),
                        stop=(sc == SC - 1),
                        skip_group_check=True,
                    )
                nc.vector.tensor_copy(out=ctxs[0:1, h * 512 : (h + 1) * 512], in_=ps_c)
            nc.sync.dma_start(out=ctx_out[b : b + 1, :], in_=ctxs)
            del state[b]

        with tc.tile_pool(name="setup", bufs=1) as setup_pool:
            setup_q_pre(setup_pool)
            load_w2(half=0)
            load(0, blocks=[0])
            load_w2(half=1)
            load(0, blocks=[1, 2, 3])
            load(1)
            load_nat(0)
            mm(0)
        for b in range(1, BPC):
            load_nat(b)
            mm(b)
            tail(b - 1)
            if b + 1 < BPC:
                load(b + 1)
        tail(BPC - 1)


_NC_CACHE = None


def build_module():
    global _NC_CACHE
    if _NC_CACHE is not None:
        return _NC_CACHE
    nc = bacc.Bacc(
        "TRN2",
        target_bir_lowering=False,
        num_devices=N_CORES,
        num_swdge_queues=2,
    )
    io = {
        "vals": nc.dram_tensor("vals", (BPC, S, D), f32, kind="ExternalInput"),
        "qry": nc.dram_tensor("qry", (BPC, D), f32, kind="ExternalInput"),
        "w1": nc.dram_tensor("w1", (D, U), f32, kind="ExternalInput"),
        "b1": nc.dram_tensor("b1", (U,), f32, kind="ExternalInput"),
        "w2": nc.dram_tensor("w2", (D, U), f32, kind="ExternalInput"),
        "b2": nc.dram_tensor("b2", (U,), f32, kind="ExternalInput"),
        "wv": nc.dram_tensor("wv", (U, 1), f32, kind="ExternalInput"),
        "bv": nc.dram_tensor("bv", (1,), f32, kind="ExternalInput"),
        "ctx_out": nc.dram_tensor("ctx_out", (BPC, D), f32, kind="ExternalOutput"),
        "aw_out": nc.dram_tensor("aw_out", (BPC, S), f32, kind="ExternalOutput"),
    }
    with tile.TileContext(nc) as tc:
        emit(nc, tc, io)
    nc.finalize()
    _NC_CACHE = nc
    return nc


def make_in_maps(query, values, w1, b1, w2, b2, wv, bv):
    query = np.asarray(query, dtype=np.float32)
    values = np.asarray(values, dtype=np.float32)
    shared = {
        "w1": np.asarray(w1, dtype=np.float32),
        "b1": np.asarray(b1, dtype=np.float32),
        "w2": np.asarray(w2, dtype=np.float32),
        "b2": np.asarray(b2, dtype=np.float32),
        "wv": np.asarray(wv, dtype=np.float32),
        "bv": np.asarray(bv, dtype=np.float32),
    }
    in_maps = []
    for c in range(N_CORES):
        sl = slice(c * BPC, (c + 1) * BPC)
        in_maps.append(
            {
                "vals": np.ascontiguousarray(values[sl]),
                "qry": np.ascontiguousarray(query[sl]),
                **shared,
            }
        )
    return in_maps


def kernel(query, values, w1, b1, w2, b2, wv, bv):
    nc = build_module()
    in_maps = make_in_maps(query, values, w1, b1, w2, b2, wv, bv)
    res = run_bass_kernel_spmd(nc, in_maps, core_ids=list(range(N_CORES)))
    ctx = np.concatenate([res.results[c]["ctx_out"] for c in range(N_CORES)], axis=0)
    aw = np.concatenate([res.results[c]["aw_out"] for c in range(N_CORES)], axis=0)
    return ctx.astype(np.float32), aw[..., None].astype(np.float32)
